# revision 29
# baseline (speedup 1.0000x reference)
"""GATv2 (nn_GATv2_49108656062978) Trainium2 Bass kernel, 8 NeuronCores SPMD.

Slot-ordered streaming architecture (v2 — replaces the dma_gather design,
whose SWDGE descriptor generation serialized ~9.4ns/edge on the Pool
engine and dominated the runtime):

  - Nodes partitioned by dst ownership: core r owns nodes [r*6250, (r+1)*6250).
    Per core, nodes are sorted by in-degree and grouped into 49 buckets of
    128 (partition dim). Bucket b has a shared compile-time slot count J[b]
    (max degree in that bucket across all cores; padding factor ~1.02).
  - The HOST lays out x source-features in slot order: for bucket b, slot j,
    the 128 columns are x[src(node_p, j)] (fp16, [F, 128*J[b]] blocks).
    Dead slots get a crafted column lam*v with a^T leaky(Wl(lam v)) << 0 for
    every head, so after exp they contribute exactly 0 in fp16 — no masks.
  - The DEVICE streams each 128-slot chunk through the PE once:
    psum[slot,256] = x_chunk^T @ [aab*Wl_perm | Ws_cmaj] (fp16 in, fp32 acc).
    No table in DRAM, no gather, no descriptor generation — the only DMAs
    are sequential streams (x-slots in, output out).
  - Edge pipeline per bucket (nodes on partitions, slots on free dim):
    DVE adds xr (score-side target transform, biases folded in) onto the
    xl half with fp16 output, Act applies Prelu in place (|att| pre-scaled
    columns, sign-split P/N reduction trick), DVE reduces per head, DVE
    subtracts N from P, Act exponentiates directly (scores are bounded,
    max|score| ~ 3.5, so no segment-max subtraction is needed), DVE reduces
    the denominator, multiplies the xs half by the weights (c-major layout
    for 2x DVE), pairwise tree-sums over slots, divides, adds bias, DMA out.
Host does graph partitioning / slot layout / small-weight reshaping and the
final unpermute. All FLOPs of the module run on device.
"""
import sys

sys.path.insert(0, "/opt/trn_rl_repo")

import numpy as np

import concourse.bass as bass
import concourse.bacc as bacc
import concourse.tile as tile
from concourse import mybir
from concourse.bass_utils import run_bass_kernel_spmd

N = 50000
F = 128
H = 4
C = 32
HC = H * C
NEG = 0.2
NCORES = 8
NPC = N // NCORES          # 6250 nodes per core
NB = (NPC + 127) // 128    # 49 buckets
NPAD = NB * 128            # 6272
LAM = 2000.0               # dead-slot column scale

f32 = mybir.dt.float32
f16 = mybir.dt.float16

LAST_RESULT = None
RUN_KWARGS = {}
G = 4                      # matmul chunks per PSUM tile (2 banks)
FLUSH = 12                 # buckets per batched output-stage flush
DEN_VIA_ACT = False        # denominator via Act exp accum_out (measured
                           # slower: 4 serial strided exps + accum reads
                           # add ~90us Act and lengthen the bucket chain)


def _find_dead_v(Wl, att):
    """v with sum_c att_hc * leaky((Wl v)_c) < -0.1 for every head, so a
    lam*v source column yields exp(score) == 0 in fp16 for any xr."""
    rng = np.random.default_rng(0)
    for _ in range(20000):
        v = rng.standard_normal(F).astype(np.float32)
        u = Wl @ v
        lu = np.where(u > 0, u, NEG * u)
        S = (lu.reshape(H, C) * att).sum(-1)
        if S.max() < -0.1:
            return v
    raise RuntimeError("no dead vector found")


def _prep(x, edge_index, Wl, bl, Wr, br, Ws, bs, att, bias):
    src = np.concatenate([edge_index[0], np.arange(N)]).astype(np.int64)
    dst = np.concatenate([edge_index[1], np.arange(N)]).astype(np.int64)
    owner = dst // NPC

    # ---- weights / att folding ----
    aflat = att.reshape(HC)
    colperm = []
    Ph = []
    for h in range(H):
        a_h = aflat[h * C:(h + 1) * C]
        pos = np.where(a_h > 0)[0]
        neg = np.where(a_h <= 0)[0]
        colperm += list(h * C + pos) + list(h * C + neg)
        Ph.append(int(len(pos)))
    colperm = np.array(colperm)
    aab = np.abs(aflat)[colperm].astype(np.float32)
    Wl_eff = aab[:, None] * Wl[colperm]
    bl_eff = aab * bl[colperm]
    Wr_eff = aab[:, None] * Wr[colperm]
    br_eff = aab * br[colperm]

    # xs stored c-major (new col k = (c=k//H, h=k%H)) so the alpha-weighting
    # multiply is innermost-contiguous on both operands (2x DVE mode).
    cmaj = np.array([(k % H) * C + k // H for k in range(HC)])
    Ws_cm = Ws[cmaj]
    # biases fold out of the slot table: bl_eff + br_eff ride on xr; bs rides
    # on the output bias (softmax weights sum to 1).
    w_it = np.ascontiguousarray(
        np.concatenate([Wl_eff.T, Ws_cm.T], axis=1), dtype=np.float16)     # [F, 256]
    wr_t = np.ascontiguousarray(Wr_eff.T, dtype=np.float16)                # [F, HC]
    br_rep = np.tile((br_eff + bl_eff)[None, :], (128, 1)).astype(np.float32)
    bout_rep = np.tile((bias + bs)[cmaj][None, :], (128, 1)).astype(np.float32)

    # ---- dead column, fp16 x with dead row appended ----
    xd = LAM * _find_dead_v(Wl, att)
    x16 = np.concatenate([x, xd[None, :]], axis=0).astype(np.float16)  # [N+1, F]

    # ---- per-core bucket packing + shared slot schedule ----
    percore = []
    Js = np.zeros((NCORES, NB), np.int64)
    for r in range(NCORES):
        sel = owner == r
        s_r = src[sel]
        d_r = dst[sel] - r * NPC
        deg = np.bincount(d_r, minlength=NPC)
        order = np.argsort(-deg, kind="stable")
        sd = deg[order]
        for b in range(NB):
            Js[r, b] = sd[b * 128:(b + 1) * 128].max()
        percore.append((order, deg, s_r, d_r))
    J = Js.max(0)
    # super-buckets: pairs of adjacent buckets share one padded width so the
    # device can process two buckets per instruction
    NS = (NB + 1) // 2
    Jsup = np.zeros(NS, np.int64)
    for s in range(NS):
        Jsup[s] = J[2 * s:2 * s + 2].max()
    Juse = np.array([Jsup[b // 2] for b in range(NB)])
    SLOTS = int(128 * Juse.sum())
    Jmax = int(Juse.max())

    in_maps = []
    orders = []
    for r in range(NCORES):
        order, deg, s_r, d_r = percore[r]
        orders.append(order)
        bp = np.empty(NPC, np.int64)
        bp[order] = np.arange(NPC)

        A = np.full((NPAD, Jmax), N, np.int64)      # default: dead column
        og = np.argsort(d_r, kind="stable")
        sg = s_r[og]
        dg = d_r[og]
        start = np.zeros(NPC + 1, np.int64)
        start[1:] = np.cumsum(deg)
        pos = np.arange(len(dg)) - start[dg]
        A[bp[dg], pos] = sg

        cols = np.concatenate(
            [A[b * 128:(b + 1) * 128, :Juse[b]].T.reshape(-1)
             for b in range(NB)])
        assert len(cols) == SLOTS
        xslot = np.ascontiguousarray(x16[cols, :].T)            # [F, SLOTS]

        xperm = np.zeros((NPAD, F), np.float16)
        xperm[:NPC] = x16[r * NPC + order]
        xperm_t = np.ascontiguousarray(xperm.T)                 # [F, NPAD]

        in_maps.append({
            "xslot": xslot, "xperm_t": xperm_t,
            "w_it": w_it, "wr_t": wr_t,
            "br_rep": br_rep, "bout_rep": bout_rep,
        })
    return in_maps, orders, Jsup, SLOTS, Ph


def _build(J, SLOTS, Ph):
    nc = bacc.Bacc("TRN2", target_bir_lowering=False, debug=False,
                   num_devices=NCORES)
    add = mybir.AluOpType.add
    sub = mybir.AluOpType.subtract
    mult = mybir.AluOpType.mult

    xslot_d = nc.dram_tensor("xslot", [F, SLOTS], f16, kind="ExternalInput")
    xperm_d = nc.dram_tensor("xperm_t", [F, NPAD], f16, kind="ExternalInput")
    w_it_d = nc.dram_tensor("w_it", [F, 256], f16, kind="ExternalInput")
    wr_t_d = nc.dram_tensor("wr_t", [F, HC], f16, kind="ExternalInput")
    br_rep_d = nc.dram_tensor("br_rep", [128, HC], f32, kind="ExternalInput")
    bout_d = nc.dram_tensor("bout_rep", [128, HC], f32, kind="ExternalInput")

    out_d = nc.dram_tensor("outp", [NPAD, HC], f32, kind="ExternalOutput")

    with nc.allow_low_precision(reason="fp16 edge pipeline; fp32 accum"), \
         tile.TileContext(nc) as tc:
        with (
            tc.tile_pool(name="const", bufs=1) as cpool,
            tc.tile_pool(name="xpool", bufs=2) as xpool,
            tc.tile_pool(name="gpool", bufs=2) as gpool,
            tc.tile_pool(name="spool", bufs=2) as spool,
            tc.tile_pool(name="psx", bufs=2, space="PSUM") as psx,
            tc.tile_pool(name="psm", bufs=3, space="PSUM") as psm,
        ):
            # ---- constants ----
            w_it_sb = cpool.tile([F, 256], f16)
            nc.sync.dma_start(w_it_sb[:], w_it_d[:])
            wr_t_sb = cpool.tile([F, HC], f16)
            nc.sync.dma_start(wr_t_sb[:], wr_t_d[:])
            br_rep_sb = cpool.tile([128, HC], f32)
            nc.sync.dma_start(br_rep_sb[:], br_rep_d[:])
            bout_sb = cpool.tile([128, HC], f32)
            nc.sync.dma_start(bout_sb[:], bout_d[:])
            xr_sb = cpool.tile([128, NB, HC], f32)
            den_all = cpool.tile([128, NB, H], f16)
            agg_all = cpool.tile([128, NB, HC], f16)

            # ---- phase X: xr per bucket, kept in SBUF (fp32) ----
            for b in range(NB):
                xpc = xpool.tile([128, 128], f16, tag="xpc")    # [f, n]
                nc.sync.dma_start(xpc[:], xperm_d[:, b * 128:(b + 1) * 128])
                pr = psx.tile([128, HC], f32)
                nc.tensor.matmul(pr[:], lhsT=xpc[:], rhs=wr_t_sb[:],
                                 start=True, stop=True)
                nc.vector.tensor_tensor(out=xr_sb[:, b, :],
                                        in0=pr[:], in1=br_rep_sb[:], op=add)
                del pr

            # ---- phase M: super-bucket loop (2 buckets per tile) ----
            need_memset_P = any(p == 0 for p in Ph)
            need_memset_N = any(p == C for p in Ph)
            NS = len(J)
            off = 0
            fstart = 0
            for s in range(NS):
                b0 = 2 * s
                nh = min(2, NB - b0)
                Jh = int(J[s])
                JS = nh * Jh
                xg = xpool.tile([128, JS * 128], f16, tag="xg")
                nc.sync.dma_start(xg[:], xslot_d[:, off:off + JS * 128])
                off += JS * 128

                g = gpool.tile([128, JS, 256], f16, tag="g")
                for j0 in range(0, JS, G):
                    gn = min(G, JS - j0)
                    ps = psm.tile([128, G * 256], f32, tag="ps")
                    for k in range(gn):
                        nc.tensor.matmul(
                            ps[:, k * 256:(k + 1) * 256],
                            lhsT=xg[:, (j0 + k) * 128:(j0 + k + 1) * 128],
                            rhs=w_it_sb[:], start=True, stop=True)
                    psv = ps[:].rearrange("p (g d) -> p g d", d=256)[:, 0:gn, :]
                    # E = xl + xr per half-run (a group may straddle the
                    # bucket boundary inside the super)
                    r0 = j0
                    while r0 < j0 + gn:
                        t = r0 // Jh
                        r1 = min((t + 1) * Jh, j0 + gn)
                        nc.vector.tensor_tensor(
                            out=g[:, r0:r1, 0:HC],
                            in0=psv[:, r0 - j0:r1 - j0, 0:HC],
                            in1=xr_sb[:, b0 + t, :].unsqueeze(1).broadcast_to(
                                [128, r1 - r0, HC]),
                            op=add)
                        r0 = r1
                    nc.scalar.activation(g[:, j0:j0 + gn, 0:HC],
                                         g[:, j0:j0 + gn, 0:HC],
                                         mybir.ActivationFunctionType.Prelu,
                                         alpha=NEG)
                    nc.scalar.copy(g[:, j0:j0 + gn, HC:256],
                                   psv[:, :, HC:256])
                    del ps

                # ---- scores (flat over both halves) ----
                scrP = spool.tile([128, JS, H], f16, tag="scrP")
                scrN = spool.tile([128, JS, H], f16, tag="scrN")
                if need_memset_P:
                    nc.vector.memset(scrP[:], 0.0)
                if need_memset_N:
                    nc.vector.memset(scrN[:], 0.0)
                h = 0
                while h < H:
                    m = 1
                    while h + m < H and Ph[h + m] == Ph[h]:
                        m += 1
                    ph = Ph[h]
                    gv = g[:, :, h * C:(h + m) * C].rearrange(
                        "p j (m c) -> p j m c", c=C)
                    if ph > 0:
                        nc.vector.tensor_reduce(
                            out=scrP[:, :, h:h + m], in_=gv[:, :, :, 0:ph],
                            axis=mybir.AxisListType.X, op=add)
                    if ph < C:
                        nc.vector.tensor_reduce(
                            out=scrN[:, :, h:h + m], in_=gv[:, :, :, ph:C],
                            axis=mybir.AxisListType.X, op=add)
                    h += m
                scr = spool.tile([128, JS, H], f16, tag="scr")
                nc.vector.tensor_tensor(out=scr[:], in0=scrP[:], in1=scrN[:],
                                        op=sub)
                pex = spool.tile([128, JS, H], f16, tag="pex")
                nc.scalar.activation(pex[:], scr[:],
                                     mybir.ActivationFunctionType.Exp)
                nc.vector.tensor_reduce(
                    out=den_all[:, b0:b0 + nh, :],
                    in_=pex[:].rearrange("p (t j) h -> p t h j", t=nh),
                    axis=mybir.AxisListType.X, op=add)

                # weighted xs in place, then per-half pairwise tree-sum
                nc.vector.tensor_tensor(
                    out=g[:, :, HC:256].rearrange("p j (c h) -> p j c h", h=H),
                    in0=g[:, :, HC:256].rearrange("p j (c h) -> p j c h", h=H),
                    in1=pex[:].unsqueeze(2).broadcast_to([128, JS, C, H]),
                    op=mult)
                gx = g[:, :, HC:256].rearrange("p (t j) d -> p t j d", t=nh)
                n = Jh
                while n > 1:
                    k = n // 2
                    dst = (agg_all[:, b0:b0 + nh, :].rearrange(
                               "p t (j d) -> p t j d", j=1)
                           if n == 2 else gx[:, :, 0:k, :])
                    nc.vector.tensor_tensor(
                        out=dst, in0=gx[:, :, 0:k, :],
                        in1=gx[:, :, n - k:n, :], op=add)
                    n = n - k

                b_last = b0 + nh - 1
                if b_last == NB - 1 or (b_last + 1) % FLUSH == 0:
                    m2 = b_last + 1 - fstart
                    rdf = spool.tile([128, FLUSH, H], f16, tag="rdf")
                    nc.vector.reciprocal(
                        rdf[:, 0:m2, :].rearrange("p m h -> p (m h)"),
                        den_all[:, fstart:b_last + 1, :].rearrange(
                            "p m h -> p (m h)"))
                    onf = spool.tile([128, FLUSH, HC], f16, tag="onf")
                    nc.vector.tensor_tensor(
                        out=onf[:, 0:m2, :].rearrange(
                            "p m (c h) -> p m c h", h=H),
                        in0=agg_all[:, fstart:b_last + 1, :].rearrange(
                            "p m (c h) -> p m c h", h=H),
                        in1=rdf[:, 0:m2, :].unsqueeze(2).broadcast_to(
                            [128, m2, C, H]),
                        op=mult)
                    obf = spool.tile([128, FLUSH, HC], f32, tag="obf")
                    nc.vector.tensor_tensor(
                        out=obf[:, 0:m2, :], in0=onf[:, 0:m2, :],
                        in1=bout_sb[:].unsqueeze(1).broadcast_to(
                            [128, m2, HC]),
                        op=add)
                    nc.sync.dma_start(
                        out_d[fstart * 128:(b_last + 1) * 128, :].rearrange(
                            "(m p) c -> p m c", p=128),
                        obf[:, 0:m2, :])
                    fstart = b_last + 1

    nc.compile()
    return nc


def kernel(**inputs) -> np.ndarray:
    global LAST_RESULT
    ins = {k: np.asarray(v) for k, v in inputs.items()}
    in_maps, orders, J, SLOTS, Ph = _prep(
        ins["x"].astype(np.float32), ins["edge_index"],
        ins["Wl"].astype(np.float32), ins["bl"].astype(np.float32),
        ins["Wr"].astype(np.float32), ins["br"].astype(np.float32),
        ins["Ws"].astype(np.float32), ins["bs"].astype(np.float32),
        ins["att"].astype(np.float32), ins["bias"].astype(np.float32))
    nc = _build(J, SLOTS, Ph)
    res = run_bass_kernel_spmd(nc, in_maps, core_ids=list(range(NCORES)),
                               **RUN_KWARGS)
    LAST_RESULT = res
    cmaj = np.array([(k % H) * C + k // H for k in range(HC)])
    inv = np.empty(HC, np.int64)
    inv[cmaj] = np.arange(HC)
    out = np.zeros((N, HC), np.float32)
    for r in range(NCORES):
        o = res.results[r]["outp"]
        out[r * NPC + orders[r]] = o[:NPC][:, inv]
    return out


# revision 30
# speedup vs baseline: 1.0191x; 1.0191x over previous
"""GATv2 (nn_GATv2_49108656062978) Trainium2 Bass kernel, 8 NeuronCores SPMD.

Slot-ordered streaming architecture (v2 — replaces the dma_gather design,
whose SWDGE descriptor generation serialized ~9.4ns/edge on the Pool
engine and dominated the runtime):

  - Nodes partitioned by dst ownership: core r owns nodes [r*6250, (r+1)*6250).
    Per core, nodes are sorted by in-degree and grouped into 49 buckets of
    128 (partition dim). Bucket b has a shared compile-time slot count J[b]
    (max degree in that bucket across all cores; padding factor ~1.02).
  - The HOST lays out x source-features in slot order: for bucket b, slot j,
    the 128 columns are x[src(node_p, j)] (fp16, [F, 128*J[b]] blocks).
    Dead slots get a crafted column lam*v with a^T leaky(Wl(lam v)) << 0 for
    every head, so after exp they contribute exactly 0 in fp16 — no masks.
  - The DEVICE streams each 128-slot chunk through the PE once:
    psum[slot,256] = x_chunk^T @ [aab*Wl_perm | Ws_cmaj] (fp16 in, fp32 acc).
    No table in DRAM, no gather, no descriptor generation — the only DMAs
    are sequential streams (x-slots in, output out).
  - Edge pipeline per bucket (nodes on partitions, slots on free dim):
    DVE adds xr (score-side target transform, biases folded in) onto the
    xl half with fp16 output, Act applies Prelu in place (|att| pre-scaled
    columns, sign-split P/N reduction trick), DVE reduces per head, DVE
    subtracts N from P, Act exponentiates directly (scores are bounded,
    max|score| ~ 3.5, so no segment-max subtraction is needed), DVE reduces
    the denominator, multiplies the xs half by the weights (c-major layout
    for 2x DVE), pairwise tree-sums over slots, divides, adds bias, DMA out.
Host does graph partitioning / slot layout / small-weight reshaping and the
final unpermute. All FLOPs of the module run on device.
"""
import sys

sys.path.insert(0, "/opt/trn_rl_repo")

import numpy as np

import concourse.bass as bass
import concourse.bacc as bacc
import concourse.tile as tile
from concourse import mybir
from concourse.bass_utils import run_bass_kernel_spmd

N = 50000
F = 128
H = 4
C = 32
HC = H * C
NEG = 0.2
NCORES = 8
NPC = N // NCORES          # 6250 nodes per core
NB = (NPC + 127) // 128    # 49 buckets
NPAD = NB * 128            # 6272
LAM = 2000.0               # dead-slot column scale

f32 = mybir.dt.float32
f16 = mybir.dt.float16

LAST_RESULT = None
RUN_KWARGS = {}
G = 4                      # matmul chunks per PSUM tile (2 banks)
FLUSH = 12                 # buckets per batched output-stage flush
DEN_VIA_ACT = False        # denominator via Act exp accum_out (measured
                           # slower: 4 serial strided exps + accum reads
                           # add ~90us Act and lengthen the bucket chain)


def _find_dead_v(Wl, att):
    """v with sum_c att_hc * leaky((Wl v)_c) < -0.1 for every head, so a
    lam*v source column yields exp(score) == 0 in fp16 for any xr."""
    rng = np.random.default_rng(0)
    for _ in range(20000):
        v = rng.standard_normal(F).astype(np.float32)
        u = Wl @ v
        lu = np.where(u > 0, u, NEG * u)
        S = (lu.reshape(H, C) * att).sum(-1)
        if S.max() < -0.1:
            return v
    raise RuntimeError("no dead vector found")


def _prep(x, edge_index, Wl, bl, Wr, br, Ws, bs, att, bias):
    src = np.concatenate([edge_index[0], np.arange(N)]).astype(np.int64)
    dst = np.concatenate([edge_index[1], np.arange(N)]).astype(np.int64)
    owner = dst // NPC

    # ---- weights / att folding ----
    aflat = att.reshape(HC)
    colperm = []
    Ph = []
    for h in range(H):
        a_h = aflat[h * C:(h + 1) * C]
        pos = np.where(a_h > 0)[0]
        neg = np.where(a_h <= 0)[0]
        colperm += list(h * C + pos) + list(h * C + neg)
        Ph.append(int(len(pos)))
    colperm = np.array(colperm)
    aab = np.abs(aflat)[colperm].astype(np.float32)
    Wl_eff = aab[:, None] * Wl[colperm]
    bl_eff = aab * bl[colperm]
    Wr_eff = aab[:, None] * Wr[colperm]
    br_eff = aab * br[colperm]

    # xs stored c-major (new col k = (c=k//H, h=k%H)) so the alpha-weighting
    # multiply is innermost-contiguous on both operands (2x DVE mode).
    cmaj = np.array([(k % H) * C + k // H for k in range(HC)])
    Ws_cm = Ws[cmaj]
    # biases fold out of the slot table: bl_eff + br_eff ride on xr; bs rides
    # on the output bias (softmax weights sum to 1).
    w_it = np.ascontiguousarray(
        np.concatenate([Wl_eff.T, Ws_cm.T], axis=1), dtype=np.float16)     # [F, 256]
    wr_t = np.ascontiguousarray(Wr_eff.T, dtype=np.float16)                # [F, HC]
    br_rep = np.tile((br_eff + bl_eff)[None, :], (128, 1)).astype(np.float32)
    bout_rep = np.tile((bias + bs)[cmaj][None, :], (128, 1)).astype(np.float32)

    # ---- dead column, fp16 x with dead row appended ----
    xd = LAM * _find_dead_v(Wl, att)
    x16 = np.concatenate([x, xd[None, :]], axis=0).astype(np.float16)  # [N+1, F]

    # ---- per-core bucket packing + shared slot schedule ----
    percore = []
    Js = np.zeros((NCORES, NB), np.int64)
    for r in range(NCORES):
        sel = owner == r
        s_r = src[sel]
        d_r = dst[sel] - r * NPC
        deg = np.bincount(d_r, minlength=NPC)
        order = np.argsort(-deg, kind="stable")
        sd = deg[order]
        for b in range(NB):
            Js[r, b] = sd[b * 128:(b + 1) * 128].max()
        percore.append((order, deg, s_r, d_r))
    J = Js.max(0)
    SLOTS = int(128 * J.sum())
    Jmax = int(J.max())

    in_maps = []
    orders = []
    for r in range(NCORES):
        order, deg, s_r, d_r = percore[r]
        orders.append(order)
        bp = np.empty(NPC, np.int64)
        bp[order] = np.arange(NPC)

        A = np.full((NPAD, Jmax), N, np.int64)      # default: dead column
        og = np.argsort(d_r, kind="stable")
        sg = s_r[og]
        dg = d_r[og]
        start = np.zeros(NPC + 1, np.int64)
        start[1:] = np.cumsum(deg)
        pos = np.arange(len(dg)) - start[dg]
        A[bp[dg], pos] = sg

        cols = np.concatenate(
            [A[b * 128:(b + 1) * 128, :J[b]].T.reshape(-1) for b in range(NB)])
        assert len(cols) == SLOTS
        xslot = np.ascontiguousarray(x16[cols, :].T)            # [F, SLOTS]

        xperm = np.zeros((NPAD, F), np.float16)
        xperm[:NPC] = x16[r * NPC + order]
        xperm_t = np.ascontiguousarray(xperm.T)                 # [F, NPAD]

        in_maps.append({
            "xslot": xslot, "xperm_t": xperm_t,
            "w_it": w_it, "wr_t": wr_t,
            "br_rep": br_rep, "bout_rep": bout_rep,
        })
    return in_maps, orders, J, SLOTS, Ph


def _build(J, SLOTS, Ph):
    nc = bacc.Bacc("TRN2", target_bir_lowering=False, debug=False,
                   num_devices=NCORES)
    add = mybir.AluOpType.add
    sub = mybir.AluOpType.subtract
    mult = mybir.AluOpType.mult

    xslot_d = nc.dram_tensor("xslot", [F, SLOTS], f16, kind="ExternalInput")
    xperm_d = nc.dram_tensor("xperm_t", [F, NPAD], f16, kind="ExternalInput")
    w_it_d = nc.dram_tensor("w_it", [F, 256], f16, kind="ExternalInput")
    wr_t_d = nc.dram_tensor("wr_t", [F, HC], f16, kind="ExternalInput")
    br_rep_d = nc.dram_tensor("br_rep", [128, HC], f32, kind="ExternalInput")
    bout_d = nc.dram_tensor("bout_rep", [128, HC], f32, kind="ExternalInput")

    out_d = nc.dram_tensor("outp", [NPAD, HC], f32, kind="ExternalOutput")

    with nc.allow_low_precision(reason="fp16 edge pipeline; fp32 accum"), \
         tile.TileContext(nc) as tc:
        with (
            tc.tile_pool(name="const", bufs=1) as cpool,
            tc.tile_pool(name="xpool", bufs=3) as xpool,
            tc.tile_pool(name="gpool", bufs=3) as gpool,
            tc.tile_pool(name="spool", bufs=3) as spool,
            tc.tile_pool(name="psx", bufs=2, space="PSUM") as psx,
            tc.tile_pool(name="psm", bufs=3, space="PSUM") as psm,
        ):
            # ---- constants ----
            w_it_sb = cpool.tile([F, 256], f16)
            nc.sync.dma_start(w_it_sb[:], w_it_d[:])
            wr_t_sb = cpool.tile([F, HC], f16)
            nc.sync.dma_start(wr_t_sb[:], wr_t_d[:])
            br_rep_sb = cpool.tile([128, HC], f32)
            nc.sync.dma_start(br_rep_sb[:], br_rep_d[:])
            bout_sb = cpool.tile([128, HC], f32)
            nc.sync.dma_start(bout_sb[:], bout_d[:])
            xr_sb = cpool.tile([128, NB, HC], f32)
            den_all = cpool.tile([128, NB, H], f16)
            agg_all = cpool.tile([128, NB, HC], f16)

            # ---- phase X: xr per bucket, kept in SBUF (fp32) ----
            for b in range(NB):
                xpc = xpool.tile([128, 128], f16, tag="xpc")    # [f, n]
                nc.sync.dma_start(xpc[:], xperm_d[:, b * 128:(b + 1) * 128])
                pr = psx.tile([128, HC], f32)
                nc.tensor.matmul(pr[:], lhsT=xpc[:], rhs=wr_t_sb[:],
                                 start=True, stop=True)
                nc.vector.tensor_tensor(out=xr_sb[:, b, :],
                                        in0=pr[:], in1=br_rep_sb[:], op=add)
                del pr

            # ---- phase M: main bucket loop ----
            need_memset_P = any(p == 0 for p in Ph)
            need_memset_N = any(p == C for p in Ph)
            off = 0
            for b in range(NB):
                Jb = int(J[b])
                xg = xpool.tile([128, Jb * 128], f16, tag="xg")
                nc.sync.dma_start(xg[:], xslot_d[:, off:off + Jb * 128])
                off += Jb * 128

                g = gpool.tile([128, Jb, 256], f16, tag="g")
                xr_b = xr_sb[:, b, :]
                for j0 in range(0, Jb, G):
                    gn = min(G, Jb - j0)
                    ps = psm.tile([128, G * 256], f32, tag="ps")
                    for k in range(gn):
                        nc.tensor.matmul(
                            ps[:, k * 256:(k + 1) * 256],
                            lhsT=xg[:, (j0 + k) * 128:(j0 + k + 1) * 128],
                            rhs=w_it_sb[:], start=True, stop=True)
                    psv = ps[:, 0:gn * 256].rearrange("p (g d) -> p g d", d=256)
                    # E = xl + xr (fp32 psum in -> fp16 out), prelu in place
                    nc.vector.tensor_tensor(
                        out=g[:, j0:j0 + gn, 0:HC], in0=psv[:, :, 0:HC],
                        in1=xr_b.unsqueeze(1).broadcast_to([128, gn, HC]),
                        op=add)
                    nc.scalar.activation(g[:, j0:j0 + gn, 0:HC],
                                         g[:, j0:j0 + gn, 0:HC],
                                         mybir.ActivationFunctionType.Prelu,
                                         alpha=NEG)
                    # xs half: drain psum on Act
                    nc.scalar.copy(g[:, j0:j0 + gn, HC:256], psv[:, :, HC:256])
                    del ps

                # ---- scores ----
                scrP = spool.tile([128, Jb, H], f16, tag="scrP")
                scrN = spool.tile([128, Jb, H], f16, tag="scrN")
                if need_memset_P:
                    nc.vector.memset(scrP[:], 0.0)
                if need_memset_N:
                    nc.vector.memset(scrN[:], 0.0)
                # merge reduces across consecutive heads with equal P-count
                # (4D AP: head blocks spaced C apart, uniform inner width)
                h = 0
                while h < H:
                    m = 1
                    while h + m < H and Ph[h + m] == Ph[h]:
                        m += 1
                    ph = Ph[h]
                    gv = g[:, :, h * C:(h + m) * C].rearrange(
                        "p j (m c) -> p j m c", c=C)
                    if ph > 0:
                        nc.vector.tensor_reduce(
                            out=scrP[:, :, h:h + m], in_=gv[:, :, :, 0:ph],
                            axis=mybir.AxisListType.X, op=add)
                    if ph < C:
                        nc.vector.tensor_reduce(
                            out=scrN[:, :, h:h + m], in_=gv[:, :, :, ph:C],
                            axis=mybir.AxisListType.X, op=add)
                    h += m
                scr = spool.tile([128, Jb, H], f16, tag="scr")
                nc.vector.tensor_tensor(out=scr[:], in0=scrP[:], in1=scrN[:],
                                        op=sub)
                pex = spool.tile([128, Jb, H], f16, tag="pex")
                den = den_all[:, b, :]
                if DEN_VIA_ACT:
                    # per-head exp; the Act accumulator yields the softmax
                    # denominator for free (sum over j of exp(scr_h))
                    for h2 in range(H):
                        nc.scalar.activation(
                            pex[:, :, h2], scr[:, :, h2],
                            mybir.ActivationFunctionType.Exp,
                            accum_out=den[:, h2:h2 + 1])
                else:
                    nc.scalar.activation(pex[:], scr[:],
                                         mybir.ActivationFunctionType.Exp)
                    nc.vector.tensor_reduce(
                        out=den[:], in_=pex[:].rearrange("p j h -> p h j"),
                        axis=mybir.AxisListType.X, op=add)

                # weighted xs in place (c-major: both operands innermost-
                # contiguous -> 2x), then pairwise tree-sum over j
                nc.vector.tensor_tensor(
                    out=g[:, :, HC:256].rearrange("p j (c h) -> p j c h", h=H),
                    in0=g[:, :, HC:256].rearrange("p j (c h) -> p j c h", h=H),
                    in1=pex[:].unsqueeze(2).broadcast_to([128, Jb, C, H]),
                    op=mult)
                n = Jb
                while n > 1:
                    k = n // 2
                    dst = (agg_all[:, b, :].rearrange("p (j d) -> p j d", j=1)
                           if n == 2 else g[:, 0:k, HC:256])
                    nc.vector.tensor_tensor(
                        out=dst, in0=g[:, 0:k, HC:256],
                        in1=g[:, n - k:n, HC:256], op=add)
                    n = n - k

                if b == NB - 1 or (b + 1) % FLUSH == 0:
                    b0 = (b // FLUSH) * FLUSH
                    m = b + 1 - b0
                    rdf = spool.tile([128, FLUSH, H], f16, tag="rdf")
                    nc.vector.reciprocal(
                        rdf[:, 0:m, :].rearrange("p m h -> p (m h)"),
                        den_all[:, b0:b + 1, :].rearrange("p m h -> p (m h)"))
                    onf = spool.tile([128, FLUSH, HC], f16, tag="onf")
                    nc.vector.tensor_tensor(
                        out=onf[:, 0:m, :].rearrange(
                            "p m (c h) -> p m c h", h=H),
                        in0=agg_all[:, b0:b + 1, :].rearrange(
                            "p m (c h) -> p m c h", h=H),
                        in1=rdf[:, 0:m, :].unsqueeze(2).broadcast_to(
                            [128, m, C, H]),
                        op=mult)
                    obf = spool.tile([128, FLUSH, HC], f32, tag="obf")
                    nc.vector.tensor_tensor(
                        out=obf[:, 0:m, :], in0=onf[:, 0:m, :],
                        in1=bout_sb[:].unsqueeze(1).broadcast_to([128, m, HC]),
                        op=add)
                    nc.sync.dma_start(
                        out_d[b0 * 128:(b + 1) * 128, :].rearrange(
                            "(m p) c -> p m c", p=128),
                        obf[:, 0:m, :])

    nc.compile()
    return nc


def kernel(**inputs) -> np.ndarray:
    global LAST_RESULT
    ins = {k: np.asarray(v) for k, v in inputs.items()}
    in_maps, orders, J, SLOTS, Ph = _prep(
        ins["x"].astype(np.float32), ins["edge_index"],
        ins["Wl"].astype(np.float32), ins["bl"].astype(np.float32),
        ins["Wr"].astype(np.float32), ins["br"].astype(np.float32),
        ins["Ws"].astype(np.float32), ins["bs"].astype(np.float32),
        ins["att"].astype(np.float32), ins["bias"].astype(np.float32))
    nc = _build(J, SLOTS, Ph)
    res = run_bass_kernel_spmd(nc, in_maps, core_ids=list(range(NCORES)),
                               **RUN_KWARGS)
    LAST_RESULT = res
    cmaj = np.array([(k % H) * C + k // H for k in range(HC)])
    inv = np.empty(HC, np.int64)
    inv[cmaj] = np.arange(HC)
    out = np.zeros((N, HC), np.float32)
    for r in range(NCORES):
        o = res.results[r]["outp"]
        out[r * NPC + orders[r]] = o[:NPC][:, inv]
    return out


# revision 32
# speedup vs baseline: 1.0291x; 1.0098x over previous
"""GATv2 (nn_GATv2_49108656062978) Trainium2 Bass kernel, 8 NeuronCores SPMD.

Slot-ordered streaming architecture (v2 — replaces the dma_gather design,
whose SWDGE descriptor generation serialized ~9.4ns/edge on the Pool
engine and dominated the runtime):

  - Nodes partitioned by dst ownership: core r owns nodes [r*6250, (r+1)*6250).
    Per core, nodes are sorted by in-degree and grouped into 49 buckets of
    128 (partition dim). Bucket b has a shared compile-time slot count J[b]
    (max degree in that bucket across all cores; padding factor ~1.02).
  - The HOST lays out x source-features in slot order: for bucket b, slot j,
    the 128 columns are x[src(node_p, j)] (fp16, [F, 128*J[b]] blocks).
    Dead slots get a crafted column lam*v with a^T leaky(Wl(lam v)) << 0 for
    every head, so after exp they contribute exactly 0 in fp16 — no masks.
  - The DEVICE streams each 128-slot chunk through the PE once:
    psum[slot,256] = x_chunk^T @ [aab*Wl_perm | Ws_cmaj] (fp16 in, fp32 acc).
    No table in DRAM, no gather, no descriptor generation — the only DMAs
    are sequential streams (x-slots in, output out).
  - Edge pipeline per bucket (nodes on partitions, slots on free dim):
    DVE adds xr (score-side target transform, biases folded in) onto the
    xl half with fp16 output, Act applies Prelu in place (|att| pre-scaled
    columns, sign-split P/N reduction trick), DVE reduces per head, DVE
    subtracts N from P, Act exponentiates directly (scores are bounded,
    max|score| ~ 3.5, so no segment-max subtraction is needed), DVE reduces
    the denominator, multiplies the xs half by the weights (c-major layout
    for 2x DVE), pairwise tree-sums over slots, divides, adds bias, DMA out.
Host does graph partitioning / slot layout / small-weight reshaping and the
final unpermute. All FLOPs of the module run on device.
"""
import sys

sys.path.insert(0, "/opt/trn_rl_repo")

import numpy as np

import concourse.bass as bass
import concourse.bacc as bacc
import concourse.tile as tile
from concourse import mybir
from concourse.bass_utils import run_bass_kernel_spmd

N = 50000
F = 128
H = 4
C = 32
HC = H * C
NEG = 0.2
NCORES = 8
NPC = N // NCORES          # 6250 nodes per core
NB = (NPC + 127) // 128    # 49 buckets
NPAD = NB * 128            # 6272
LAM = 2000.0               # dead-slot column scale

f32 = mybir.dt.float32
f16 = mybir.dt.float16

LAST_RESULT = None
RUN_KWARGS = {}
G = 4                      # matmul chunks per PSUM tile (2 banks)
FLUSH = 12                 # buckets per batched output-stage flush
DEN_VIA_ACT = False        # denominator via Act exp accum_out (measured
                           # slower: 4 serial strided exps + accum reads
                           # add ~90us Act and lengthen the bucket chain)


def _find_dead_v(Wl, att):
    """v with sum_c att_hc * leaky((Wl v)_c) < -0.1 for every head, so a
    lam*v source column yields exp(score) == 0 in fp16 for any xr."""
    rng = np.random.default_rng(0)
    for _ in range(20000):
        v = rng.standard_normal(F).astype(np.float32)
        u = Wl @ v
        lu = np.where(u > 0, u, NEG * u)
        S = (lu.reshape(H, C) * att).sum(-1)
        if S.max() < -0.1:
            return v
    raise RuntimeError("no dead vector found")


def _prep(x, edge_index, Wl, bl, Wr, br, Ws, bs, att, bias):
    src = np.concatenate([edge_index[0], np.arange(N)]).astype(np.int64)
    dst = np.concatenate([edge_index[1], np.arange(N)]).astype(np.int64)
    owner = dst // NPC

    # ---- weights / att folding ----
    aflat = att.reshape(HC)
    colperm = []
    Ph = []
    for h in range(H):
        a_h = aflat[h * C:(h + 1) * C]
        pos = np.where(a_h > 0)[0]
        neg = np.where(a_h <= 0)[0]
        colperm += list(h * C + pos) + list(h * C + neg)
        Ph.append(int(len(pos)))
    colperm = np.array(colperm)
    aab = np.abs(aflat)[colperm].astype(np.float32)
    Wl_eff = aab[:, None] * Wl[colperm]
    bl_eff = aab * bl[colperm]
    Wr_eff = aab[:, None] * Wr[colperm]
    br_eff = aab * br[colperm]

    # xs stored c-major (new col k = (c=k//H, h=k%H)) so the alpha-weighting
    # multiply is innermost-contiguous on both operands (2x DVE mode).
    cmaj = np.array([(k % H) * C + k // H for k in range(HC)])
    Ws_cm = Ws[cmaj]
    # biases fold out of the slot table: bl_eff + br_eff ride on xr; bs rides
    # on the output bias (softmax weights sum to 1).
    w_it = np.ascontiguousarray(
        np.concatenate([Wl_eff.T, Ws_cm.T], axis=1), dtype=np.float16)     # [F, 256]
    wr_t = np.ascontiguousarray(Wr_eff.T, dtype=np.float16)                # [F, HC]
    br_rep = np.tile((br_eff + bl_eff)[None, :], (128, 1)).astype(np.float32)
    bout_rep = np.tile((bias + bs)[cmaj][None, :], (128, 1)).astype(np.float32)

    # ---- dead column, fp16 x with dead row appended ----
    xd = LAM * _find_dead_v(Wl, att)
    x16 = np.concatenate([x, xd[None, :]], axis=0).astype(np.float16)  # [N+1, F]

    # ---- per-core bucket packing + shared slot schedule ----
    percore = []
    Js = np.zeros((NCORES, NB), np.int64)
    for r in range(NCORES):
        sel = owner == r
        s_r = src[sel]
        d_r = dst[sel] - r * NPC
        deg = np.bincount(d_r, minlength=NPC)
        order = np.argsort(-deg, kind="stable")
        sd = deg[order]
        for b in range(NB):
            Js[r, b] = sd[b * 128:(b + 1) * 128].max()
        percore.append((order, deg, s_r, d_r))
    J = Js.max(0)
    SLOTS = int(128 * J.sum())
    Jmax = int(J.max())

    in_maps = []
    orders = []
    for r in range(NCORES):
        order, deg, s_r, d_r = percore[r]
        orders.append(order)
        bp = np.empty(NPC, np.int64)
        bp[order] = np.arange(NPC)

        A = np.full((NPAD, Jmax), N, np.int64)      # default: dead column
        og = np.argsort(d_r, kind="stable")
        sg = s_r[og]
        dg = d_r[og]
        start = np.zeros(NPC + 1, np.int64)
        start[1:] = np.cumsum(deg)
        pos = np.arange(len(dg)) - start[dg]
        A[bp[dg], pos] = sg

        cols = np.concatenate(
            [A[b * 128:(b + 1) * 128, :J[b]].T.reshape(-1) for b in range(NB)])
        assert len(cols) == SLOTS
        xslot = np.ascontiguousarray(x16[cols, :].T)            # [F, SLOTS]

        xperm = np.zeros((NPAD, F), np.float16)
        xperm[:NPC] = x16[r * NPC + order]
        xperm_t = np.ascontiguousarray(xperm.T)                 # [F, NPAD]

        in_maps.append({
            "xslot": xslot, "xperm_t": xperm_t,
            "w_it": w_it, "wr_t": wr_t,
            "br_rep": br_rep, "bout_rep": bout_rep,
        })
    return in_maps, orders, J, SLOTS, Ph


def _build(J, SLOTS, Ph):
    nc = bacc.Bacc("TRN2", target_bir_lowering=False, debug=False,
                   num_devices=NCORES)
    add = mybir.AluOpType.add
    sub = mybir.AluOpType.subtract
    mult = mybir.AluOpType.mult

    xslot_d = nc.dram_tensor("xslot", [F, SLOTS], f16, kind="ExternalInput")
    xperm_d = nc.dram_tensor("xperm_t", [F, NPAD], f16, kind="ExternalInput")
    w_it_d = nc.dram_tensor("w_it", [F, 256], f16, kind="ExternalInput")
    wr_t_d = nc.dram_tensor("wr_t", [F, HC], f16, kind="ExternalInput")
    br_rep_d = nc.dram_tensor("br_rep", [128, HC], f32, kind="ExternalInput")
    bout_d = nc.dram_tensor("bout_rep", [128, HC], f32, kind="ExternalInput")

    out_d = nc.dram_tensor("outp", [NPAD, HC], f32, kind="ExternalOutput")

    with nc.allow_low_precision(reason="fp16 edge pipeline; fp32 accum"), \
         tile.TileContext(nc) as tc:
        with (
            tc.tile_pool(name="const", bufs=1) as cpool,
            tc.tile_pool(name="xpool", bufs=3) as xpool,
            tc.tile_pool(name="gpool", bufs=3) as gpool,
            tc.tile_pool(name="spool", bufs=3) as spool,
            tc.tile_pool(name="psx", bufs=2, space="PSUM") as psx,
            tc.tile_pool(name="psm", bufs=3, space="PSUM") as psm,
        ):
            # ---- constants ----
            w_it_sb = cpool.tile([F, 256], f16)
            nc.sync.dma_start(w_it_sb[:], w_it_d[:])
            wr_t_sb = cpool.tile([F, HC], f16)
            nc.sync.dma_start(wr_t_sb[:], wr_t_d[:])
            br_rep_sb = cpool.tile([128, HC], f32)
            nc.sync.dma_start(br_rep_sb[:], br_rep_d[:])
            bout_sb = cpool.tile([128, HC], f32)
            nc.sync.dma_start(bout_sb[:], bout_d[:])
            xr_sb = cpool.tile([128, NB, HC], f32)
            agg_all = cpool.tile([128, NB, HC + H], f16)

            # ---- phase X: xr per bucket, kept in SBUF (fp32) ----
            for b in range(NB):
                xpc = xpool.tile([128, 128], f16, tag="xpc")    # [f, n]
                nc.sync.dma_start(xpc[:], xperm_d[:, b * 128:(b + 1) * 128])
                pr = psx.tile([128, HC], f32)
                nc.tensor.matmul(pr[:], lhsT=xpc[:], rhs=wr_t_sb[:],
                                 start=True, stop=True)
                nc.vector.tensor_tensor(out=xr_sb[:, b, :],
                                        in0=pr[:], in1=br_rep_sb[:], op=add)
                del pr

            # ---- phase M: main bucket loop ----
            need_memset_P = any(p == 0 for p in Ph)
            need_memset_N = any(p == C for p in Ph)
            off = 0
            for b in range(NB):
                Jb = int(J[b])
                xg = xpool.tile([128, Jb * 128], f16, tag="xg")
                nc.sync.dma_start(xg[:], xslot_d[:, off:off + Jb * 128])
                off += Jb * 128

                g = gpool.tile([128, Jb, 256 + H], f16, tag="g")
                # ones ride the aggregation: their alpha-weighted tree-sum
                # IS the softmax denominator (no separate den reduce)
                nc.vector.memset(g[:, :, 256:256 + H], 1.0)
                xr_b = xr_sb[:, b, :]
                for j0 in range(0, Jb, G):
                    gn = min(G, Jb - j0)
                    ps = psm.tile([128, G * 256], f32, tag="ps")
                    for k in range(gn):
                        nc.tensor.matmul(
                            ps[:, k * 256:(k + 1) * 256],
                            lhsT=xg[:, (j0 + k) * 128:(j0 + k + 1) * 128],
                            rhs=w_it_sb[:], start=True, stop=True)
                    psv = ps[:, 0:gn * 256].rearrange("p (g d) -> p g d", d=256)
                    # E = xl + xr (fp32 psum in -> fp16 out), prelu in place
                    nc.vector.tensor_tensor(
                        out=g[:, j0:j0 + gn, 0:HC], in0=psv[:, :, 0:HC],
                        in1=xr_b.unsqueeze(1).broadcast_to([128, gn, HC]),
                        op=add)
                    nc.scalar.activation(g[:, j0:j0 + gn, 0:HC],
                                         g[:, j0:j0 + gn, 0:HC],
                                         mybir.ActivationFunctionType.Prelu,
                                         alpha=NEG)
                    # xs half: drain psum on Act
                    nc.scalar.copy(g[:, j0:j0 + gn, HC:256], psv[:, :, HC:256])
                    del ps

                # ---- scores ----
                scrP = spool.tile([128, Jb, H], f16, tag="scrP")
                scrN = spool.tile([128, Jb, H], f16, tag="scrN")
                if need_memset_P:
                    nc.vector.memset(scrP[:], 0.0)
                if need_memset_N:
                    nc.vector.memset(scrN[:], 0.0)
                # merge reduces across consecutive heads with equal P-count
                # (4D AP: head blocks spaced C apart, uniform inner width)
                h = 0
                while h < H:
                    m = 1
                    while h + m < H and Ph[h + m] == Ph[h]:
                        m += 1
                    ph = Ph[h]
                    gv = g[:, :, h * C:(h + m) * C].rearrange(
                        "p j (m c) -> p j m c", c=C)
                    if ph > 0:
                        nc.vector.tensor_reduce(
                            out=scrP[:, :, h:h + m], in_=gv[:, :, :, 0:ph],
                            axis=mybir.AxisListType.X, op=add)
                    if ph < C:
                        nc.vector.tensor_reduce(
                            out=scrN[:, :, h:h + m], in_=gv[:, :, :, ph:C],
                            axis=mybir.AxisListType.X, op=add)
                    h += m
                scr = spool.tile([128, Jb, H], f16, tag="scr")
                nc.vector.tensor_tensor(out=scr[:], in0=scrP[:], in1=scrN[:],
                                        op=sub)
                pex = spool.tile([128, Jb, H], f16, tag="pex")
                nc.scalar.activation(pex[:], scr[:],
                                     mybir.ActivationFunctionType.Exp)

                # weighted xs in place (c-major: both operands innermost-
                # contiguous -> 2x), then pairwise tree-sum over j
                nc.vector.tensor_tensor(
                    out=g[:, :, HC:256 + H].rearrange(
                        "p j (c h) -> p j c h", h=H),
                    in0=g[:, :, HC:256 + H].rearrange(
                        "p j (c h) -> p j c h", h=H),
                    in1=pex[:].unsqueeze(2).broadcast_to([128, Jb, C + 1, H]),
                    op=mult)
                n = Jb
                while n > 1:
                    k = n // 2
                    dst = (agg_all[:, b, :].rearrange("p (j d) -> p j d", j=1)
                           if n == 2 else g[:, 0:k, HC:256 + H])
                    nc.vector.tensor_tensor(
                        out=dst, in0=g[:, 0:k, HC:256 + H],
                        in1=g[:, n - k:n, HC:256 + H], op=add)
                    n = n - k

                if b == NB - 1 or (b + 1) % FLUSH == 0:
                    b0 = (b // FLUSH) * FLUSH
                    m = b + 1 - b0
                    rdf = spool.tile([128, FLUSH, H], f16, tag="rdf")
                    nc.vector.reciprocal(
                        rdf[:, 0:m, :], agg_all[:, b0:b + 1, HC:HC + H])
                    onf = spool.tile([128, FLUSH, HC], f16, tag="onf")
                    nc.vector.tensor_tensor(
                        out=onf[:, 0:m, :].rearrange(
                            "p m (c h) -> p m c h", h=H),
                        in0=agg_all[:, b0:b + 1, 0:HC].rearrange(
                            "p m (c h) -> p m c h", h=H),
                        in1=rdf[:, 0:m, :].unsqueeze(2).broadcast_to(
                            [128, m, C, H]),
                        op=mult)
                    obf = spool.tile([128, FLUSH, HC], f32, tag="obf")
                    nc.vector.tensor_tensor(
                        out=obf[:, 0:m, :], in0=onf[:, 0:m, :],
                        in1=bout_sb[:].unsqueeze(1).broadcast_to([128, m, HC]),
                        op=add)
                    nc.sync.dma_start(
                        out_d[b0 * 128:(b + 1) * 128, :].rearrange(
                            "(m p) c -> p m c", p=128),
                        obf[:, 0:m, :])

    nc.compile()
    return nc


def kernel(**inputs) -> np.ndarray:
    global LAST_RESULT
    ins = {k: np.asarray(v) for k, v in inputs.items()}
    in_maps, orders, J, SLOTS, Ph = _prep(
        ins["x"].astype(np.float32), ins["edge_index"],
        ins["Wl"].astype(np.float32), ins["bl"].astype(np.float32),
        ins["Wr"].astype(np.float32), ins["br"].astype(np.float32),
        ins["Ws"].astype(np.float32), ins["bs"].astype(np.float32),
        ins["att"].astype(np.float32), ins["bias"].astype(np.float32))
    nc = _build(J, SLOTS, Ph)
    res = run_bass_kernel_spmd(nc, in_maps, core_ids=list(range(NCORES)),
                               **RUN_KWARGS)
    LAST_RESULT = res
    cmaj = np.array([(k % H) * C + k // H for k in range(HC)])
    inv = np.empty(HC, np.int64)
    inv[cmaj] = np.arange(HC)
    out = np.zeros((N, HC), np.float32)
    for r in range(NCORES):
        o = res.results[r]["outp"]
        out[r * NPC + orders[r]] = o[:NPC][:, inv]
    return out
